# revision 7
# baseline (speedup 1.0000x reference)
# Trainium2 Bass kernel for masked (key-padding) attention layer.
#
#   q,k,v = x@Wq, x@Wk, x@Wv ; score = q@k^T/sqrt(T) masked over keys;
#   out = softmax(score)@v @ Wo
#
# Sharding: data-parallel over batch, B=8 -> one batch element per NeuronCore,
# weights broadcast on-device via AllGather (host ships one row-shard per core).
#
# The dominant cost in this deployment is the host<->device tunnel (~60-90
# MB/s) plus per-call jax dispatch, NOT device execution (~150us). So the
# kernel is organized to minimize bytes moved per call and to reuse one cached
# jitted executable:
#   - x ships once, bf16, in its natural [T, D] layout (no host transpose);
#     the device transposes it with the PE array (matmul-with-identity).
#   - the folded weights (A = Wq@Wk^T, Avo = Wv@Wo, computed on host: 2 tiny
#     512^3 GEMMs) ship row-SHARDED (1/8th per core) and are AllGathered
#     on-device over NeuronLink instead of being replicated over the tunnel.
#   - the output is written bf16 in natural [T, D] layout (stage C emits the
#     [t, o] orientation directly) and cast to f32 on host.
#   - the donated "zero" output operands the bass custom-call needs are
#     device-resident: first call materializes them with a tiny on-device jit;
#     later calls recycle the previous call's (already copied) output buffer.
#
# Per-core algorithm (everything keyed off the pre-folded weights):
#   xT = transpose(x)                       (PE transpose, 64 128x128 blocks)
#   u[x,j]  = sum_x' A[x,x'] xT[x',j]       (64 MMs)
#   v2[j,o] = sum_x xT[x,j] Avo[x,o]        (64 MMs)
#   sT[j,t] = sum_x u[x,j] xT[x,t]          (256 MMs)
#   eT = exp(sT/sqrt(T) + kbias)            (ScalarE, PSUM->SBUF bf16)
#   den[t] = sum_j eT[j,t] -> gpsimd partition all-reduce -> reciprocal
#   eT *= recip (DVE)  ; out[t,o] = sum_j eT[j,t] v2[j,o]   (256 MMs, natural
#   orientation: lhsT=eT chunk, rhs=v2 -> PSUM [t, o] -> bf16 -> DMA out)
import math

import numpy as np
import ml_dtypes

B = 8
T = 2048
D = 512
P = 128
KC = D // P       # 4 contraction chunks of 128
QB = 512          # free-dim chunk (one PSUM bank of f32)
NQ = T // QB      # 4 query chunks
NT = T // P       # 16 tiles of 128
WSH = D // B      # 64-row weight shard per core
SCALE = 1.0 / math.sqrt(float(T))
PAD_BIAS = -30000.0

_BF16 = ml_dtypes.bfloat16

_ctx: dict = {}


def _build():
    """Build + compile the single-core SPMD program (mask-independent)."""
    import concourse.bass as bass
    import concourse.bass_isa as bass_isa
    import concourse.mybir as mybir
    import concourse.tile as tile
    from concourse import bacc

    dt = mybir.dt
    f32, bf16 = dt.float32, dt.bfloat16

    nc = bacc.Bacc(
        "TRN2",
        target_bir_lowering=False,
        debug=False,
        enable_asserts=False,
        num_devices=B,
    )

    i8 = dt.int8

    xin_d = nc.dram_tensor("xin", [T, D], bf16, kind="ExternalInput")
    ws_d = nc.dram_tensor("ws", [WSH, D], bf16, kind="ExternalInput")
    vs_d = nc.dram_tensor("vs", [WSH, D], bf16, kind="ExternalInput")
    kbias_d = nc.dram_tensor("kbias", [P, NT], f32, kind="ExternalInput")
    ident_d = nc.dram_tensor("ident", [P, P], bf16, kind="ExternalInput")
    # out rows 0..T-1: per-row int8 quantized output; rows T..T+15: the
    # per-row f32 scales (absmax/127), bit-packed (scl[p, n] = scale of row
    # t = n*128+p).
    out_d = nc.dram_tensor("out", [T + NT, D], i8, kind="ExternalOutput")

    Exp = mybir.ActivationFunctionType.Exp

    with tile.TileContext(nc) as tc:
        with (
            tc.tile_pool(name="const", bufs=1) as cpool,
            tc.tile_pool(name="big", bufs=1) as bpool,
            tc.tile_pool(name="psum", bufs=6, space="PSUM") as psum,
            tc.tile_pool(name="psumt", bufs=2, space="PSUM") as psumt,
            tc.tile_pool(name="outs", bufs=4) as opool,
            tc.tile_pool(name="dram", bufs=1, space="DRAM") as dram,
        ):
            # ---- persistent SBUF tensors ----
            xrow = bpool.tile([P, NT, D], bf16, tag="xrow")
            xT = bpool.tile([P, KC, T], bf16, tag="xT")
            AT = cpool.tile([P, KC, D], bf16, tag="AT")
            Avo = cpool.tile([P, KC, D], bf16, tag="Avo")
            kbias = cpool.tile([P, NT], f32, tag="kbias")
            ident = cpool.tile([P, P], bf16, tag="ident")
            u = bpool.tile([P, KC, T], bf16, tag="u")
            v2 = bpool.tile([P, NT, D], bf16, tag="v2")
            eT = bpool.tile([P, NT, T], bf16, tag="eT")
            dacc = bpool.tile([P, T], f32, tag="dacc")
            rbc = bpool.tile([P, T], f32, tag="rbc")

            # ---- weight shards: DRAM->DRAM bounce, AllGather over
            # NeuronLink, then load gathered [D, D] into SBUF. Issued first
            # so the comm overlaps the x load + transpose.
            cinA = dram.tile([WSH, D], bf16, name="cinA")
            coutA = dram.tile([D, D], bf16, name="coutA")
            cinV = dram.tile([WSH, D], bf16, name="cinV")
            coutV = dram.tile([D, D], bf16, name="coutV")
            nc.gpsimd.dma_start(cinA[:], ws_d.ap())
            nc.gpsimd.dma_start(cinV[:], vs_d.ap())
            nc.gpsimd.collective_compute(
                "AllGather",
                mybir.AluOpType.bypass,
                replica_groups=[list(range(B))],
                ins=[cinA.opt()],
                outs=[coutA.opt()],
            )
            nc.gpsimd.collective_compute(
                "AllGather",
                mybir.AluOpType.bypass,
                replica_groups=[list(range(B))],
                ins=[cinV.opt()],
                outs=[coutV.opt()],
            )
            nc.sync.dma_start(ident[:], ident_d.ap())
            nc.sync.dma_start(kbias[:], kbias_d.ap())
            nc.sync.dma_start(
                xrow[:], xin_d.ap().rearrange("(n p) d -> p n d", p=P)
            )
            nc.sync.dma_start(AT[:], coutA.rearrange("(c p) h -> p c h", p=P))
            nc.sync.dma_start(Avo[:], coutV.rearrange("(c p) h -> p c h", p=P))
            nc.vector.memset(dacc[:], 0.0)

            # ---- stage T: xT = x^T via PE transpose, 4 blocks per copy ----
            for c in range(KC):
                for nb in range(0, NT, 4):
                    pt = psumt.tile([P, 4 * P], bf16, tag="pt", name="pt")
                    for i in range(4):
                        nc.tensor.transpose(
                            pt[:, i * P : (i + 1) * P],
                            xrow[:, nb + i, c * P : (c + 1) * P],
                            ident[:],
                        )
                    nc.vector.tensor_copy(
                        xT[:, c, nb * P : (nb + 4) * P], pt[:]
                    )

            # ---- stage A1: u = A @ x^T  [x, j] ----
            for jc in range(NQ):
                pk = [psum.tile([P, QB], f32, tag="ps", name="ps")
                      for _ in range(KC)]
                for c in range(KC):
                    for m in range(KC):
                        nc.tensor.matmul(
                            pk[m][:],
                            AT[:, c, m * P : (m + 1) * P],
                            xT[:, c, jc * QB : (jc + 1) * QB],
                            start=(c == 0),
                            stop=(c == KC - 1),
                        )
                for m in range(KC):
                    nc.vector.tensor_copy(
                        u[:, m, jc * QB : (jc + 1) * QB], pk[m][:]
                    )

            # ---- stage A2: v2 = x @ Avo  [j, o] ----
            for j in range(NT):
                pv = psum.tile([P, D], f32, tag="ps", name="ps")
                for c in range(KC):
                    nc.tensor.matmul(
                        pv[:],
                        xT[:, c, j * P : (j + 1) * P],
                        Avo[:, c, :],
                        start=(c == 0),
                        stop=(c == KC - 1),
                    )
                nc.vector.tensor_copy(v2[:, j, :], pv[:])

            # ---- stage B: scores + exp + denominator accumulation ----
            for j in range(NT):
                ps = [psum.tile([P, QB], f32, tag="ps", name="ps")
                      for _ in range(NQ)]
                for c in range(KC):
                    for t in range(NQ):
                        nc.tensor.matmul(
                            ps[t][:],
                            u[:, c, j * P : (j + 1) * P],
                            xT[:, c, t * QB : (t + 1) * QB],
                            start=(c == 0),
                            stop=(c == KC - 1),
                        )
                for t in range(NQ):
                    sl = slice(t * QB, (t + 1) * QB)
                    nc.scalar.activation(
                        eT[:, j, sl],
                        ps[t][:],
                        Exp,
                        bias=kbias[:, j : j + 1],
                        scale=SCALE,
                    )
                    nc.vector.tensor_add(dacc[:, sl], dacc[:, sl], eT[:, j, sl])

            # ---- denominator: gpsimd all-reduce across partitions puts the
            # broadcast column sums of dacc directly in rbc; reciprocal.
            for tt in range(NQ):
                sl = slice(tt * QB, (tt + 1) * QB)
                nc.gpsimd.partition_all_reduce(
                    rbc[:, sl], dacc[:, sl], P, bass_isa.ReduceOp.add
                )
                nc.vector.reciprocal(rbc[:, sl], rbc[:, sl])

            # ---- normalize attention weights in place (so stage C's output
            # needs no per-query scaling in the [t, o] orientation).
            for j in range(NT):
                nc.vector.tensor_mul(eT[:, j, :], eT[:, j, :], rbc[:])

            # ---- stage C: out[t,o] = sum_j eN[j,t] v2[j,o] in natural
            # orientation (lhsT = eT chunk, rhs = v2); each [128, D] tile is
            # quantized to int8 with a per-row (per query) scale and streams
            # to DRAM as soon as it completes.
            scl = bpool.tile([P, NT], f32, tag="scl")
            for tt in range(NT):
                po = psum.tile([P, D], f32, tag="ps", name="ps")
                for j in range(NT):
                    nc.tensor.matmul(
                        po[:],
                        eT[:, j, tt * P : (tt + 1) * P],
                        v2[:, j, :],
                        start=(j == 0),
                        stop=(j == NT - 1),
                    )
                mx = opool.tile([P, 1], f32, tag="mx", name="mx")
                rq = opool.tile([P, 1], f32, tag="rq", name="rq")
                nc.vector.tensor_reduce(
                    mx[:], po[:], mybir.AxisListType.X,
                    mybir.AluOpType.max, apply_absolute_value=True,
                )
                nc.vector.tensor_scalar_max(mx[:], mx[:], 1e-30)
                nc.vector.reciprocal(rq[:], mx[:])
                nc.vector.tensor_scalar_mul(rq[:], rq[:], 127.0)
                nc.vector.tensor_scalar_mul(
                    scl[:, tt : tt + 1], mx[:], 1.0 / 127.0
                )
                ot = opool.tile([P, D], i8, tag="ot", name="ot")
                nc.vector.tensor_mul(ot[:], po[:], rq[:].broadcast_to([P, D]))
                nc.sync.dma_start(out_d[tt * P : (tt + 1) * P, :], ot[:])
            # scales: [P, NT] f32 = 64 bytes/partition -> 16 int8 rows
            nc.sync.dma_start(
                out_d[T : T + NT, :].rearrange("a (q m) -> (a q) m", m=64),
                scl[:].bitcast(i8),
            )

    nc.compile()
    return nc


def _get_ctx():
    """Build the program and a cached jitted executable (once per process)."""
    if "run" in _ctx:
        return _ctx
    import jax
    import jax.numpy as jnp
    from jax.experimental.shard_map import shard_map
    from jax.sharding import Mesh, PartitionSpec, NamedSharding
    import concourse.mybir as mybir
    from concourse import bass2jax

    bass2jax.install_neuronx_cc_hook()
    nc = _build()
    partition_name = nc.partition_id_tensor.name if nc.partition_id_tensor else None
    in_names, out_names, out_avals = [], [], []
    for alloc in nc.m.functions[0].allocations:
        if not isinstance(alloc, mybir.MemoryLocationSet):
            continue
        name = alloc.memorylocations[0].name
        if alloc.kind == "ExternalInput":
            if name != partition_name:
                in_names.append(name)
        elif alloc.kind == "ExternalOutput":
            out_names.append(name)
            shape = tuple(alloc.tensor_shape)
            dtype = mybir.dt.np(alloc.dtype)
            out_avals.append(jax.core.ShapedArray(shape, dtype))
    n_params = len(in_names)
    n_outs = len(out_avals)
    all_names = list(in_names) + out_names
    if partition_name is not None:
        all_names = all_names + [partition_name]
    donate = tuple(range(n_params, n_params + n_outs))

    def _body(*args):
        operands = list(args)
        if partition_name is not None:
            operands.append(bass2jax.partition_id_tensor())
        outs = bass2jax._bass_exec_p.bind(
            *operands,
            out_avals=tuple(out_avals),
            in_names=tuple(all_names),
            out_names=tuple(out_names),
            lowering_input_output_aliases=(),
            sim_require_finite=True,
            sim_require_nnan=True,
            nc=nc,
        )
        return tuple(outs)

    devices = jax.devices()[:B]
    mesh = Mesh(np.asarray(devices), ("core",))
    in_specs = (PartitionSpec("core"),) * (n_params + n_outs)
    out_specs = (PartitionSpec("core"),) * n_outs
    sharded = jax.jit(
        shard_map(_body, mesh=mesh, in_specs=in_specs, out_specs=out_specs,
                  check_rep=False),
        donate_argnums=donate,
        keep_unused=True,
    )

    csh = NamedSharding(mesh, PartitionSpec("core"))
    zero_fn = jax.jit(
        lambda: tuple(
            jnp.zeros((B * a.shape[0],) + tuple(a.shape[1:]), a.dtype)
            for a in out_avals
        ),
        out_shardings=(csh,) * n_outs,
    )

    # identity matrix is a constant input: keep it resident on device.
    ident_np = np.tile(np.eye(P, dtype=np.float32).astype(_BF16), (B, 1))
    ident_dev = jax.device_put(ident_np, csh)

    _ctx.update(
        nc=nc,
        in_names=in_names,
        sharded=sharded,
        zero_fn=zero_fn,
        ident_dev=ident_dev,
        prev_out=None,
        run=True,
    )
    return _ctx


def _prep_args(x, mask, W_q, W_k, W_v, W_o):
    """Host-side prep: bf16 cast of x, folded weights, key-padding bias."""
    x16 = np.asarray(x, np.float32).astype(_BF16).reshape(B * T, D)
    wq = np.asarray(W_q, np.float32)
    wk = np.asarray(W_k, np.float32)
    wv = np.asarray(W_v, np.float32)
    wo = np.asarray(W_o, np.float32)
    a = wq @ wk.T          # [x, x']; score = x @ A @ x^T
    avo = wv @ wo          # [x, o];  out = attn @ x @ Avo
    ws = np.ascontiguousarray(a.T).astype(_BF16)    # [D, D] = 8 x [64, D] shards
    vs = np.ascontiguousarray(avo).astype(_BF16)
    bias = np.where(np.asarray(mask) != 0, np.float32(0.0),
                    np.float32(PAD_BIAS)).astype(np.float32)      # [B, T]
    kb = np.ascontiguousarray(
        bias.reshape(B, NT, P).transpose(0, 2, 1)
    ).reshape(B * P, NT)
    return {"xin": x16, "ws": ws, "vs": vs, "kbias": kb}


def kernel(x, mask, W_q, W_k, W_v, W_o):
    ctx = _get_ctx()
    args = _prep_args(x, mask, W_q, W_k, W_v, W_o)
    operands = []
    for name in ctx["in_names"]:
        if name == "ident":
            operands.append(ctx["ident_dev"])
        else:
            operands.append(args[name])
    if ctx["prev_out"] is not None:
        zeros = (ctx["prev_out"],)
    else:
        zeros = ctx["zero_fn"]()
    outs = ctx["sharded"](*operands, *zeros)
    raw = np.asarray(outs[0]).reshape(B, T + NT, D)
    ctx["prev_out"] = outs[0]
    q = raw[:, :T, :].astype(np.float32)
    # scales: [B, 16, 512] int8 rows -> f32 [B, 128, 16] -> per-row t = n*128+p
    scale = (
        np.ascontiguousarray(raw[:, T:, :])
        .view(np.float32)
        .reshape(B, P, NT)
        .transpose(0, 2, 1)
        .reshape(B, T, 1)
    )
    return q * scale
